# revision 45
# baseline (speedup 1.0000x reference)
"""Trainium2 Bass kernel for nn_BartDoubleTinyAttention.

Module: LayerNorm -> 1024->64 down-proj -> cross-attention (encoder KV)
        -> self-attention -> 64->1024 up-proj -> x + 0.001*h

Sharding: 8 cores = (batch b in 0..3) x (sequence half h in 0..1); each core
owns 1024 query tokens. The cross-attention numerator/denominator mix
([64 mix + r1] = [65, 1024] bf16) is summed across the two cores of a batch
pair with a 2-rank AllReduce; each core recovers the partner half by
subtracting its own. Self-attention uses per-core KV order [own || other]
(softmax is KV-permutation invariant).

Key structure (vs a straightforward port):
 - wo1/wv1 and every bias are folded on the host into composed matrices;
   the self-attention q2/k2/v2 projections read the *unnormalized* cross
   mix w1m directly, with the softmax denominator r1 riding as an extra
   contraction row so all biases stay exact.
 - The 1/r1 normalization of self-attention KV happens inside the exp via
   the Activation engine's per-partition scale/bias operands (and fused
   scalar ops on DVE tiles); only the query side needs one explicit
   broadcast-multiply.
 - exp is split between the Activation engine (table exp) and the Vector
   engine (Schraudolph: one fused tensor_scalar writing int16 bits that are
   re-read as bf16).
 - The final softmax normalization (1/r2) and the residual add are fused
   into one DVE scalar_tensor_tensor per output tile, using r2 transposed
   into per-partition columns; the up-projection bias rides the r2 row of
   the stationary so it comes out exact.
 - LayerNorm stats via DVE bn_stats/bn_aggr on the natural-layout bf16 x;
   the Act engine does a single Rsqrt; k1 biases fold via a ones-row in the
   host-fed transposed encoder.
 - Attention loops are software-pipelined (scores for chunk c+1 issue
   before the PV matmul of chunk c) so the PE never stalls on an exp.
"""

import math
from contextlib import ExitStack

import numpy as np
import ml_dtypes

B = 4
T_FULL = 2048
S_FULL = 2048
D_IN = 1024
DA = 64
SCALE = DA ** -0.5
EPS = 1e-5
RES_SCALE = 0.001
N_CORES = 8
P = 128

BF16 = ml_dtypes.bfloat16

# Schraudolph exp constants (bf16 bit space): i16 = round(s*EXPA + EXPB),
# bits reinterpreted as bf16 give exp(s) to ~3%.
EXPA = 184.6650558  # log2(e) * 2^7
EXPB = 16252.0      # 127 * 2^7 - 4 (balanced error)
# ln approximation (fp32 bit space): ln(x) ~ (i32(x) - B)*LNK + 0.03
LNK = 8.2629582e-8  # ln2 / 2^23
LNC = 88.02969193 - 0.03  # B*LNK - correction

_CACHE = {}


def _slices(total, step=512):
    out = []
    o = 0
    while o < total:
        sz = min(step, total - o)
        out.append((o, sz))
        o += sz
    return out


def build_program(t_own, s_full, d_in, groups, n_act_exp1=9, n_act_exp2=4):
    """Emit the SPMD bass program (identical on all cores)."""
    import concourse.bass as bass
    import concourse.tile as tile
    from concourse import bacc, mybir

    f32 = mybir.dt.float32
    bf16 = mybir.dt.bfloat16
    fp8 = mybir.dt.float8e4
    i16 = mybir.dt.int16
    i32 = mybir.dt.int32
    AF = mybir.ActivationFunctionType
    ALU = mybir.AluOpType

    FC = d_in // P            # feature chunks for the down-projection
    SC = s_full // P          # encoder kv chunks (cross attention)
    TC = t_own // P           # own-token chunks
    OC = t_own // P           # kv chunks per half (self attention)

    nc = bacc.Bacc("TRN2", target_bir_lowering=False)

    dp = nc.declare_dram_parameter
    x_nat = dp("x_nat", [t_own, d_in], bf16, isOutput=False)
    xT = dp("xT", [d_in, t_own], fp8, isOutput=False)
    encTa = dp("encTa", [DA + 1, s_full], bf16, isOutput=False)
    enca = dp("enca", [s_full, DA + 1], bf16, isOutput=False)
    q1s = dp("q1s", [d_in, DA], fp8, isOutput=False)
    k1s = dp("k1s", [DA + 1, DA + 2], bf16, isOutput=False)
    q2s = dp("q2s", [DA, DA], bf16, isOutput=False)
    k2s = dp("k2s", [DA + 1, DA + 1], bf16, isOutput=False)
    v2s = dp("v2s", [DA + 1, DA + 1], bf16, isOutput=False)
    outw = dp("outw", [DA + 1, d_in], bf16, isOutput=False)
    ident = dp("ident", [P, P], bf16, isOutput=False)
    out = dp("out", [t_own, d_in], f32, isOutput=True)

    with tile.TileContext(nc) as tc:
        with ExitStack() as ctx:
            sing = ctx.enter_context(tc.tile_pool(name="sing", bufs=1))
            bigx = ctx.enter_context(tc.tile_pool(name="bigx", bufs=1))
            work = ctx.enter_context(tc.tile_pool(name="work", bufs=3))
            outp = ctx.enter_context(tc.tile_pool(name="outp", bufs=3))
            once = ctx.enter_context(tc.tile_pool(name="once", bufs=2))
            ps_small = ctx.enter_context(
                tc.tile_pool(name="ps_small", bufs=2, space="PSUM"))
            ps_acc = ctx.enter_context(
                tc.tile_pool(name="ps_acc", bufs=1, space="PSUM"))
            ps_big = ctx.enter_context(
                tc.tile_pool(name="ps_big", bufs=2, space="PSUM"))
            dram = ctx.enter_context(
                tc.tile_pool(name="dram", bufs=1, space="DRAM"))

            # ---------------- weights / constants (sync queue) -----------
            sb_k1s = sing.tile([DA + 1, DA + 2], bf16)
            nc.sync.dma_start(sb_k1s[:], k1s[:])
            sb_q1s = sing.tile([P, FC, DA], fp8)
            nc.sync.dma_start(sb_q1s[:], q1s.rearrange("(c p) m -> p c m", p=P))
            sb_ident = sing.tile([P, P], bf16)
            nc.sync.dma_start(sb_ident[:], ident[:])
            sb_identf = sing.tile([P, P], f32)
            nc.vector.tensor_copy(out=sb_identf[:], in_=sb_ident[:])
            sb_eps = sing.tile([1, 1], f32)
            nc.vector.memset(sb_eps[:], EPS)
            sb_ones64 = sing.tile([1, DA], bf16)
            nc.vector.memset(sb_ones64[:], 1.0)
            sb_onecol = sing.tile([DA + 1, 1], bf16)
            nc.vector.memset(sb_onecol[:], 1.0)
            sb_one1 = sb_onecol[DA:DA + 1, :]

            # ---------------- big input loads ----------------------------
            # x_nat first (gates the deep LayerNorm-stats chain); xT in
            # per-chunk pieces so the chunk-outer down-projection can start
            # as soon as the first chunk lands.
            xr = x_nat.rearrange("(c p) d -> p c d", p=P)
            x_tiles = []
            for i in range(TC):
                xt = bigx.tile([P, d_in], bf16, tag=f"x{i}")
                nc.gpsimd.dma_start(xt[:], xr[:, i, :])
                x_tiles.append(xt)
            sb_encTa = bigx.tile([DA + 1, s_full], bf16)
            nc.sync.dma_start(sb_encTa[:], encTa[:])
            sb_xT = bigx.tile([P, FC, t_own], fp8)
            xTr = xT.rearrange("(c p) t -> p c t", p=P)
            for c in range(FC):
                nc.scalar.dma_start(sb_xT[:, c, :], xTr[:, c, :])
            sb_enca = bigx.tile([P, SC, DA + 1], bf16)
            nc.sync.dma_start(sb_enca[:],
                              enca.rearrange("(c p) d -> p c d", p=P))
            # remaining (late-use) weights after the critical loads
            sb_q2s = sing.tile([DA, DA], bf16)
            nc.sync.dma_start(sb_q2s[:], q2s[:])
            sb_k2s = sing.tile([DA + 1, DA + 1], bf16)
            nc.sync.dma_start(sb_k2s[:], k2s[:])
            sb_v2s = sing.tile([DA + 1, DA + 1], bf16)
            nc.sync.dma_start(sb_v2s[:], v2s[:])
            sb_outw = sing.tile([DA + 1, d_in], bf16)
            nc.sync.dma_start(sb_outw[:], outw[:])

            # ---------------- LayerNorm stats (bn_stats per tile) --------
            # DVE-only work; emitted first so it runs while the PE does
            # k1aug and the down-projection.
            statcols = sing.tile([P, 2 * TC], f32)
            for i in range(TC):
                bno = once.tile([P, 12], f32, tag="bno")
                nc.vector.bn_stats(bno[:, 0:6], x_tiles[i][:, 0:512])
                nc.vector.bn_stats(bno[:, 6:12], x_tiles[i][:, 512:1024])
                nc.vector.bn_aggr(statcols[:, 2 * i:2 * i + 2], bno[:])

            # ---------------- K1 keys (biases folded via ones-row) -------
            # First PE work: depends only on the small encTa load.
            k1aug = sing.tile([DA + 2, s_full], bf16)
            for (ns, nsz) in _slices(s_full):
                pk = ps_small.tile([DA + 2, nsz], f32, tag="ps_small")
                nc.tensor.matmul(pk[:], sb_k1s[:], sb_encTa[:, ns:ns + nsz],
                                 start=True, stop=True)
                nc.scalar.activation(out=k1aug[:, ns:ns + nsz], in_=pk[:],
                                     func=AF.Copy)

            # ---------------- down-projection (raw q1) -------------------
            # chunk-outer so the PE consumes xT chunks as they arrive
            ps_q1 = ps_acc.tile([DA, t_own], f32, tag="ps_acc")
            for c in range(FC):
                for (ns, nsz) in _slices(t_own):
                    nc.tensor.matmul(ps_q1[:, ns:ns + nsz], sb_q1s[:, c, :],
                                     sb_xT[:, c, ns:ns + nsz],
                                     start=(c == 0), stop=(c == FC - 1))

            # stats columns -> rows (after the down-proj in the PE queue)
            mu_row_t = sing.tile([1, t_own], f32)
            var_row_t = sing.tile([1, t_own], f32)
            for i in range(TC):
                for j, dst in ((0, mu_row_t), (1, var_row_t)):
                    pst = ps_small.tile([1, P], f32, tag="ps_small")
                    nc.tensor.transpose(
                        pst[:], statcols[:, 2 * i + j:2 * i + j + 1],
                        sb_identf[:])
                    nc.vector.tensor_copy(out=dst[:, i * P:(i + 1) * P],
                                          in_=pst[:])
            mu_row = mu_row_t[:]
            var_row = var_row_t[:]
            lgv_row = once.tile([1, t_own], f32, tag="lgv")
            nc.scalar.activation(out=lgv_row[:], in_=var_row, func=AF.Ln,
                                 bias=sb_eps[:])
            # bias -ln(256) folds away the x256 scale on the fp8 q1 weights
            rsig_row = sing.tile([1, t_own], bf16)
            sb_mln256 = sing.tile([1, 1], f32)
            nc.vector.memset(sb_mln256[:], -math.log(256.0))
            nc.scalar.activation(out=rsig_row[:], in_=lgv_row[:], func=AF.Exp,
                                 scale=-0.5, bias=sb_mln256[:])
            m2_row = sing.tile([1, t_own], bf16)
            nc.vector.tensor_mul(m2_row[:], mu_row, rsig_row[:])

            # a few warm-keepers so the PE stays at full clock while the
            # LayerNorm row chain (Ln/Exp on the Act engine) completes
            for _ in range(6):
                pw = ps_small.tile([P, 512], f32, tag="ps_small")
                nc.tensor.matmul(pw[:], sb_ident[:], x_tiles[0][:, 0:512],
                                 start=True, stop=True)

            # rsig broadcast to 64 partitions via ones-matmul + Act copy
            ps_rb = ps_big.tile([P, t_own], f32, tag="ps_big")
            for (ns, nsz) in _slices(t_own):
                nc.tensor.matmul(ps_rb[0:DA, ns:ns + nsz], sb_ones64[:],
                                 rsig_row[:, ns:ns + nsz],
                                 start=True, stop=True)
            rsig_b = sing.tile([DA, t_own], bf16)
            nc.scalar.activation(out=rsig_b[:], in_=ps_rb[0:DA, :],
                                 func=AF.Copy)

            # q1aug: rows 0-63 = rsig*q1raw, row 64 = mu*rsig, row 65 = 1
            q1aug = sing.tile([DA + 2, t_own], bf16)
            nc.vector.tensor_mul(q1aug[0:DA, :], ps_q1[:], rsig_b[:])
            nc.vector.memset(q1aug[DA:DA + 2, :], 1.0)
            nc.vector.tensor_copy(out=q1aug[DA:DA + 1, :], in_=m2_row[:])

            # ---------------- cross attention (pipelined chunks) ---------
            ps_mix = ps_acc.tile([DA + 1, t_own], f32, tag="ps_acc")
            sl = _slices(t_own)

            def scores1(c):
                ps_s = ps_big.tile([P, t_own], f32, tag="ps_big")
                for (ns, nsz) in sl:
                    nc.tensor.matmul(ps_s[:, ns:ns + nsz],
                                     k1aug[:, c * P:(c + 1) * P],
                                     q1aug[:, ns:ns + nsz],
                                     start=True, stop=True)
                return ps_s

            def exp1(ps_s, c):
                # halves on Act and DVE concurrently: halves exp latency and
                # keeps the PE stream gap-free (HAM stays un-throttled)
                a1 = work.tile([P, t_own], bf16, tag="a_t")
                h = t_own // 2
                nc.scalar.activation(out=a1[:, 0:h], in_=ps_s[:, 0:h],
                                     func=AF.Exp)
                nc.vector.tensor_scalar(
                    out=a1[:, h:].bitcast(i16), in0=ps_s[:, h:],
                    scalar1=EXPA, scalar2=EXPB,
                    op0=ALU.mult, op1=ALU.add)
                return a1

            def pv1(a1, c):
                for (ns, nsz) in sl:
                    nc.tensor.matmul(ps_mix[:, ns:ns + nsz], sb_enca[:, c, :],
                                     a1[:, ns:ns + nsz],
                                     start=(c == 0), stop=(c == SC - 1))

            prev = None
            for c in range(SC):
                ps_s = scores1(c)
                if prev is not None:
                    pv1(exp1(*prev), prev[1])
                prev = (ps_s, c)
            pv1(exp1(*prev), prev[1])

            # mix + r1 row -> bf16, exchange with partner core
            w1maug = sing.tile([DA + 1, t_own], bf16)
            nc.scalar.activation(out=w1maug[:], in_=ps_mix[:], func=AF.Copy)
            cc_in = dram.tile([DA + 1, t_own], bf16)
            cc_out = dram.tile([DA + 1, t_own], bf16)
            nc.sync.dma_start(cc_in[:], w1maug[:])
            nc.gpsimd.collective_compute(
                "AllReduce", mybir.AluOpType.add, replica_groups=groups,
                ins=[cc_in.opt()], outs=[cc_out.opt()])

            # ---------------- self attention: own-half prep --------------
            # q2 (query side, explicitly normalized by 1/r1[t]); the
            # reciprocal reads the PSUM row directly so it runs in parallel
            # with the w1maug copy above
            rc1_row = sing.tile([1, t_own], bf16)
            with nc.allow_low_precision(reason="1/r1 only scales softmax "
                                        "weights; bf16 is ample here"):
                nc.vector.reciprocal(rc1_row[:], ps_mix[DA:DA + 1, :])
            ps_q2 = ps_big.tile([P, t_own], f32, tag="ps_big")
            for (ns, nsz) in sl:
                nc.tensor.matmul(ps_q2[0:DA, ns:ns + nsz], sb_q2s[:],
                                 w1maug[0:DA, ns:ns + nsz],
                                 start=True, stop=True)
            q2aug = sing.tile([DA + 1, t_own], bf16)

            k2raw = sing.tile([DA + 1, 2 * t_own], bf16)
            v2raw = sing.tile([P, 2 * OC, DA + 1], bf16)
            sc_a = sing.tile([P, 2 * OC], f32)   # act exp scale (1/r1)
            sc_b = sing.tile([P, 2 * OC], f32)   # act exp bias (-ln r1)
            sd_a = sing.tile([P, 2 * OC], f32)   # dve exp scale (EXPA/r1)
            sd_b = sing.tile([P, 2 * OC], f32)   # dve exp bias
            LNK16 = 0.6931471805599453 / 128.0   # ln-approx slope, bf16 bits
            LNC16 = 16256 * LNK16 - 0.03

            def prep_half(src, off):
                """k2raw/v2raw + exp scale/bias columns for one half.
                src = [65, t_own] bf16 (rows 0-63 mix, row 64 r1)."""
                for (ns, nsz) in sl:
                    pk2 = ps_small.tile([DA + 1, nsz], f32, tag="ps_small")
                    nc.tensor.matmul(pk2[:], sb_k2s[:], src[:, ns:ns + nsz],
                                     start=True, stop=True)
                    nc.scalar.activation(
                        out=k2raw[:, off * t_own + ns:off * t_own + ns + nsz],
                        in_=pk2[:], func=AF.Copy)
                for c in range(OC):
                    pv2 = ps_small.tile([P, DA + 1], f32, tag="ps_small")
                    nc.tensor.matmul(pv2[:], src[:, c * P:(c + 1) * P],
                                     sb_v2s[:], start=True, stop=True)
                    nc.vector.tensor_copy(out=v2raw[:, off * OC + c, :],
                                          in_=pv2[:])
                # r1 per kv token is column 64 of v2raw (V2AUG's last col)
                cs = slice(off * OC, off * OC + OC)
                r1c = v2raw[:, cs, DA:DA + 1].squeeze()
                nc.vector.reciprocal(sc_a[:, cs], r1c)
                nc.vector.tensor_scalar(
                    out=sc_b[:, cs], in0=r1c.bitcast(i16),
                    scalar1=-LNK16, scalar2=LNC16, op0=ALU.mult, op1=ALU.add)
                nc.vector.tensor_scalar_mul(sd_a[:, cs], sc_a[:, cs], EXPA)
                nc.vector.tensor_scalar(
                    out=sd_b[:, cs], in0=sc_b[:, cs],
                    scalar1=EXPA, scalar2=EXPB, op0=ALU.mult, op1=ALU.add)

            prep_half(w1maug[:], 0)

            # fill the reciprocal->bcast->copy->mul latency chain
            for _ in range(8):
                pw = ps_small.tile([P, 512], f32, tag="ps_small")
                nc.tensor.matmul(pw[:], sb_ident[:], x_tiles[0][:, 0:512],
                                 start=True, stop=True)

            # rc1 broadcast + q2aug assembled while prep_half fills the PE;
            # the bcast reads rc1_row (DVE reciprocal) computed in parallel.
            ps_rc = ps_big.tile([P, t_own], f32, tag="ps_big")
            for (ns, nsz) in sl:
                nc.tensor.matmul(ps_rc[0:DA, ns:ns + nsz], sb_ones64[:],
                                 rc1_row[:, ns:ns + nsz],
                                 start=True, stop=True)
            rc1_b = sing.tile([DA, t_own], bf16)
            nc.scalar.activation(out=rc1_b[:], in_=ps_rc[0:DA, :],
                                 func=AF.Copy)
            nc.vector.tensor_mul(q2aug[0:DA, :], ps_q2[0:DA, :], rc1_b[:])
            nc.vector.memset(q2aug[DA:DA + 1, :], 1.0)

            ps_o2 = ps_acc.tile([DA + 1, t_own], f32, tag="ps_acc")

            def scores2(c):
                ps_s2 = ps_big.tile([P, t_own], f32, tag="ps_big")
                for (ns, nsz) in sl:
                    nc.tensor.matmul(ps_s2[:, ns:ns + nsz],
                                     k2raw[:, c * P:(c + 1) * P],
                                     q2aug[:, ns:ns + nsz],
                                     start=True, stop=True)
                return ps_s2

            def exp2(ps_s2, c, use_act):
                a2 = work.tile([P, t_own], bf16, tag="a_t")
                h = t_own // 2
                nc.scalar.activation(out=a2[:, 0:h], in_=ps_s2[:, 0:h],
                                     func=AF.Exp,
                                     scale=sc_a[:, c:c + 1],
                                     bias=sc_b[:, c:c + 1])
                nc.vector.tensor_scalar(
                    out=a2[:, h:].bitcast(i16), in0=ps_s2[:, h:],
                    scalar1=sd_a[:, c:c + 1], scalar2=sd_b[:, c:c + 1],
                    op0=ALU.mult, op1=ALU.add)
                return a2

            def pv2(a2, c):
                for (ns, nsz) in sl:
                    nc.tensor.matmul(ps_o2[:, ns:ns + nsz], v2raw[:, c, :],
                                     a2[:, ns:ns + nsz],
                                     start=(c == 0), stop=(c == 2 * OC - 1))

            def self_attn_half(cs, n_act):
                prev = None
                for j, c in enumerate(cs):
                    ps_s2 = scores2(c)
                    if prev is not None:
                        pv2(exp2(prev[0], prev[1], prev[2]), prev[1])
                    prev = (ps_s2, c, j < n_act)
                pv2(exp2(prev[0], prev[1], prev[2]), prev[1])

            self_attn_half(range(OC), n_act_exp2)

            # Keep the PE's HAM activity window busy while the AllReduce is
            # in flight: idle > ~3.4us re-throttles the clock to 1.2 GHz and
            # the whole post-collective phase would run at half speed.
            for _ in range(44):
                pw = ps_small.tile([P, 512], f32, tag="ps_small")
                nc.tensor.matmul(pw[:], sb_ident[:], x_tiles[0][:, 0:512],
                                 start=True, stop=True)

            # -------- partner half arrives: sum - own = other -------------
            sum_sb = sing.tile([DA + 1, t_own], bf16)
            nc.sync.dma_start(sum_sb[:], cc_out[:])
            w1m_oth = sing.tile([DA + 1, t_own], bf16)
            nc.vector.tensor_tensor(out=w1m_oth[:], in0=sum_sb[:],
                                    in1=w1maug[:], op=ALU.subtract)
            prep_half(w1m_oth[:], 1)
            self_attn_half(range(OC, 2 * OC), n_act_exp2)

            # ---------------- out-projection + fused residual ------------
            o2raw = sing.tile([DA + 1, t_own], bf16)
            nc.scalar.activation(out=o2raw[:], in_=ps_o2[:], func=AF.Copy)
            r2cols = sing.tile([P, TC], f32)
            for c in range(TC):
                pr = ps_small.tile([P, 1], bf16, tag="ps_small")
                nc.tensor.transpose(pr[:], o2raw[DA:DA + 1, c * P:(c + 1) * P],
                                    sb_one1)
                nc.vector.tensor_copy(out=r2cols[:, c:c + 1], in_=pr[:])
            rc2cols = sing.tile([P, TC], f32)
            nc.vector.reciprocal(rc2cols[:], r2cols[:])

            out_r = out.rearrange("(c p) d -> p c d", p=P)
            for i in range(TC):
                po = ps_big.tile([P, d_in], f32, tag="ps_big")
                for (ns, nsz) in _slices(d_in):
                    nc.tensor.matmul(po[:, ns:ns + nsz],
                                     o2raw[:, i * P:(i + 1) * P],
                                     sb_outw[:, ns:ns + nsz],
                                     start=True, stop=True)
                # normalization copy on Act (idle at tail), cheap add on DVE
                hn = outp.tile([P, d_in], bf16, tag="hn")
                nc.scalar.activation(out=hn[:], in_=po[:], func=AF.Copy,
                                     scale=rc2cols[:, i:i + 1])
                ot = outp.tile([P, d_in], f32, tag="ot")
                nc.vector.tensor_add(ot[:], hn[:], x_tiles[i][:])
                q = nc.sync if i % 2 == 0 else nc.scalar
                q.dma_start(out_r[:, i, :], ot[:])

    nc.compile()
    return nc


def prep_weights(f):
    """Host-side composition of the tiny weight matrices (all fp32 numpy)."""
    g, bl = f["ln_g"], f["ln_b"]
    d_in = f["w1"].shape[1]
    da = DA
    w1g = f["w1"] * g[None, :]
    c1 = f["w1"] @ bl + f["b1"]
    q1_w = SCALE * (f["wq1"] @ w1g)                     # [64, D]
    q1_b = SCALE * (f["wq1"] @ c1 + f["bq1"])           # [64]
    s1v = q1_w.sum(axis=1)                              # [64]

    # K1S [65, 66]: keys from [enc.T ; ones], cols: 64 keys + mean-corr +
    # bias-corr rows of the score contraction.
    k1s = np.zeros((da + 1, da + 2), np.float32)
    k1s[0:da, 0:da] = f["wk1"].T
    k1s[da, 0:da] = f["bk1"]
    k1s[0:da, da] = -(f["wk1"].T @ s1v)
    k1s[da, da] = -(f["bk1"] @ s1v)
    k1s[0:da, da + 1] = f["wk1"].T @ q1_b
    k1s[da, da + 1] = f["bk1"] @ q1_b

    # fold wo1*wv1 (and bv1/bo1) into the q2/k2/v2 path: h_mid = o1e @ M1.T
    # + m_b where o1e = softmax1 @ enc.
    M1 = f["wo1"] @ f["wv1"]                            # [64, 64]
    m_b = f["wo1"] @ f["bv1"] + f["bo1"]                # [64]
    q2_w = SCALE * (f["wq2"] @ M1)
    q2_b = SCALE * (f["wq2"] @ m_b + f["bq2"])
    k2_w = f["wk2"] @ M1
    k2_b = f["wk2"] @ m_b + f["bk2"]
    v2_w = f["wv2"] @ M1
    v2_b = f["wv2"] @ m_b + f["bv2"]

    k2s = np.zeros((da + 1, da + 1), np.float32)
    k2s[0:da, 0:da] = k2_w.T
    k2s[da, 0:da] = k2_b
    k2s[0:da, da] = k2_w.T @ q2_b
    k2s[da, da] = k2_b @ q2_b

    v2s = np.zeros((da + 1, da + 1), np.float32)
    v2s[0:da, 0:da] = v2_w.T
    v2s[da, 0:da] = v2_b
    v2s[da, da] = 1.0

    out_w = RES_SCALE * (f["w2"] @ f["wo2"])            # [D, 64]
    out_b = RES_SCALE * (f["w2"] @ f["bo2"] + f["b2"])  # [D]
    outw = np.zeros((da + 1, d_in), np.float32)
    outw[0:da, :] = out_w.T
    outw[da, :] = out_b

    bf = lambda a: np.ascontiguousarray(a).astype(BF16)
    f8 = lambda a: np.ascontiguousarray(a).astype(ml_dtypes.float8_e4m3fn)
    return {
        "q1s": f8(256.0 * q1_w.T),
        "k1s": bf(k1s),
        "q2s": bf(q2_w.T),
        "k2s": bf(k2s),
        "v2s": bf(v2s),
        "outw": bf(outw),
        "ident": bf(np.eye(P, dtype=np.float32)),
    }


def make_in_maps(inputs, t_own=T_FULL // 2):
    """Build the per-core input dicts from the full problem inputs."""
    f = {k: np.asarray(v, np.float32) for k, v in inputs.items()}
    w = prep_weights(f)
    x = f["hidden_states"]
    enc = f["encoder_hidden_states"]
    b_count = x.shape[0]
    in_maps = []
    for c in range(2 * b_count):
        b, h = c // 2, c % 2
        xo = np.ascontiguousarray(x[b, h * t_own:(h + 1) * t_own, :])
        m = dict(w)
        m["x_nat"] = xo.astype(BF16)
        m["xT"] = np.ascontiguousarray(xo.T).astype(ml_dtypes.float8_e4m3fn)
        encta = np.ones((DA + 1, enc.shape[1]), np.float32)
        encta[0:DA, :] = enc[b].T
        m["encTa"] = np.ascontiguousarray(encta).astype(BF16)
        ea = np.ones((enc.shape[1], DA + 1), np.float32)
        ea[:, 0:DA] = enc[b]
        m["enca"] = ea.astype(BF16)
        in_maps.append(m)
    return in_maps


LAST_RESULT = None


def kernel(**inputs):
    global LAST_RESULT
    from concourse.bass_utils import run_bass_kernel_spmd

    t_own = T_FULL // 2
    groups = [[0, 1], [2, 3], [4, 5], [6, 7]]
    key = (t_own, S_FULL, D_IN)
    if key not in _CACHE:
        _CACHE[key] = build_program(t_own, S_FULL, D_IN, groups)
    nc = _CACHE[key]

    in_maps = make_in_maps(inputs, t_own)
    res = run_bass_kernel_spmd(nc, in_maps, core_ids=list(range(N_CORES)))
    LAST_RESULT = res

    out = np.empty((B, T_FULL, D_IN), dtype=np.float32)
    for c in range(N_CORES):
        b, h = c // 2, c % 2
        out[b, h * t_own:(h + 1) * t_own, :] = res.results[c]["out"]
    return out
